# revision 5
# baseline (speedup 1.0000x reference)
"""Distributed Trainium2 kernel for nn_Attention_49529562858354.

Reference computation (per batch): LayerNorm(x) @ w_qkv -> 16-head
self-attention with key-side masking (mask==1 -> key excluded).

Sharding (8 cores): core = batch * 2 + head_group. Data parallel over
the 4 batches, tensor parallel over 2 groups of 8 heads. Each core gets
its batch's x, the w_qkv column slice for its heads, and produces
out[:, hg*512:(hg+1)*512] for its batch. No collectives needed.

Per-core pipeline (bf16 compute, f32 LN stats):
  1. LayerNorm in natural layout (bn_stats); kv rows also fold in the
     key gate (masked/pad keys -> 0 rows, which zeroes their V rows and
     denominator entries downstream). xhat -> bf16 -> DRAM scratch in
     row chunks -> chunked DMA-transposes -> xT [d, tokens] tiles.
     ln_g/ln_b applied per-partition on xT.
  2. QKV projections: qT/kT as [cols, tokens], v natural with a gate
     column appended per head (softmax denominator via the AV matmul).
  3. Attention by head PAIRS (row groups 0-63 / 64-127 of the PE array
     work concurrently, hiding the per-matmul weight load): scores
     transposed [k, q] in PSUM, exp on ScalarE with fused scale (no max
     subtraction: post-LN logits are O(1)), AV accumulates [65, 512]
     with row 64 = denominator. Epilogue: PE transpose, reciprocal,
     scale, one [128, 512] out tile per 128 queries.

Masked keys are removed on the host (gather) and padded to a multiple
of 128 with gate=0 rows, roughly halving attention work. Set
KERNEL_DENSE=1 to run dense (all 2048 keys, gate = 1-mask).
"""

import os
import sys
import types

for _p in ("/opt/trn_rl_repo", "/root/.axon_site"):
    if _p not in sys.path:
        sys.path.insert(0, _p)

import numpy as np
import ml_dtypes

import concourse.bass as bass
import concourse.tile as tile
from concourse import mybir

N_CORES = 8
N_TOK = 2048
DIM = 1024
HEADS_LOCAL = 8
DH = 64
COLS = HEADS_LOCAL * DH  # 512 columns per core per q/k/v
SCALE = DH ** -0.5
EPS = 1e-5
QCHUNK = 512
KGROUP = 3  # score k-tiles per PSUM group / exp call
COMPACT = os.environ.get("KERNEL_DENSE", "") != "1"

F32 = mybir.dt.float32
BF16 = mybir.dt.bfloat16
MUL = mybir.AluOpType.mult
ADD = mybir.AluOpType.add

LAST_EXEC_TIME_NS = None


def _split_excess_waits(nc, max_waits=1, max_updates=1):
    """This container's walrus rejects >1 sync wait/update per
    instruction; move overflow onto adjacent same-engine NoOps."""
    counter = [0]

    def fresh():
        counter[0] += 1
        return f"I-WFIX-{counter[0]}"

    for f in nc.m.functions:
        for blk in f.blocks:
            il = blk.instructions
            out = []
            changed = False
            for inst in il:
                si = inst.sync_info
                if si is None:
                    out.append(inst)
                    continue
                waits = list(si.on_wait or [])
                updates = list(si.on_update or [])
                pre, post = [], []
                if len(waits) > max_waits:
                    for w in waits[max_waits:]:
                        nop = mybir.InstNoOp(name=fresh(), ins=[], outs=[])
                        nop.engine = inst.engine
                        nop.sync_info = mybir.SyncInfo(on_wait=[w], on_update=[])
                        pre.append(nop)
                    waits = waits[:max_waits]
                if len(updates) > max_updates:
                    for u in updates[max_updates:]:
                        nop = mybir.InstNoOp(name=fresh(), ins=[], outs=[])
                        nop.engine = inst.engine
                        nop.sync_info = mybir.SyncInfo(on_wait=[], on_update=[u])
                        post.append(nop)
                    updates = updates[:max_updates]
                if pre or post:
                    inst.sync_info = mybir.SyncInfo(on_wait=waits, on_update=updates)
                    changed = True
                out.extend(pre)
                out.append(inst)
                out.extend(post)
            if changed:
                blk.instructions = out


def build_graph(l_kv):
    lt = l_kv // 128  # kv token tiles
    nc = bass.Bass()

    x_ext = nc.declare_dram_parameter("x", [N_TOK, DIM], F32, isOutput=False)
    xkv_ext = (
        nc.declare_dram_parameter("xkv", [l_kv, DIM], F32, isOutput=False)
        if COMPACT
        else None
    )
    gate_ext = nc.declare_dram_parameter("gate", [l_kv], F32, isOutput=False)
    gate_rep_ext = nc.declare_dram_parameter(
        "gate_rep", [128, lt * HEADS_LOCAL], F32, isOutput=False
    )
    wq_ext = nc.declare_dram_parameter("wq", [DIM, COLS], F32, isOutput=False)
    wk_ext = nc.declare_dram_parameter("wk", [DIM, COLS], F32, isOutput=False)
    wv_ext = nc.declare_dram_parameter("wv", [DIM, COLS], F32, isOutput=False)
    g_ext = nc.declare_dram_parameter("ln_g", [DIM], F32, isOutput=False)
    b_ext = nc.declare_dram_parameter("ln_b", [DIM], F32, isOutput=False)
    out_ext = nc.declare_dram_parameter("out", [N_TOK, COLS], F32, isOutput=True)

    # Row-chunked scratches so DMA-transposes pipeline with the LN.
    QCH_ROWS, NQCH = 512, N_TOK // 512  # 4 q chunks x 4 tiles
    KCH_TILES = 3
    kv_chunks = []  # (row0, nrows)
    t0 = 0
    while t0 < lt:
        n = min(KCH_TILES, lt - t0)
        kv_chunks.append((t0 * 128, n * 128))
        t0 += n
    scr_q = [
        nc.dram_tensor(f"scr_q{c}", [QCH_ROWS, DIM], BF16) for c in range(NQCH)
    ]
    scr_kv = [
        nc.dram_tensor(f"scr_kv{c}", [nr, DIM], BF16)
        for c, (_, nr) in enumerate(kv_chunks)
    ]

    with tile.TileContext(nc) as tc:
        import contextlib

        with contextlib.ExitStack() as ctx:
            singles = ctx.enter_context(tc.tile_pool(name="singles", bufs=1))
            xin = ctx.enter_context(tc.tile_pool(name="xin", bufs=3))
            stats = ctx.enter_context(tc.tile_pool(name="stats", bufs=3))
            xhat_pool = ctx.enter_context(tc.tile_pool(name="xhat", bufs=3))
            wtmp = ctx.enter_context(tc.tile_pool(name="wtmp", bufs=2))
            psum = ctx.enter_context(tc.tile_pool(name="psum", bufs=1, space="PSUM"))
            p_pool = ctx.enter_context(tc.tile_pool(name="p_sb", bufs=2))
            o_pool = ctx.enter_context(tc.tile_pool(name="o_sb", bufs=2))
            recip_pool = ctx.enter_context(tc.tile_pool(name="recip", bufs=2))

            # --- constants -------------------------------------------------
            g_sb = singles.tile([128, 8], F32, tag="g_sb")
            nc.sync.dma_start(out=g_sb[:], in_=g_ext.rearrange("(kd p) -> p kd", p=128))
            b_sb = singles.tile([128, 8], F32, tag="b_sb")
            nc.sync.dma_start(out=b_sb[:], in_=b_ext.rearrange("(kd p) -> p kd", p=128))
            gate_sb = singles.tile([128, lt], F32, tag="gate_sb")
            nc.sync.dma_start(
                out=gate_sb[:], in_=gate_ext.rearrange("(t p) -> p t", p=128)
            )
            gate_rep_sb = singles.tile([128, lt * HEADS_LOCAL], F32, tag="gate_rep_sb")
            nc.sync.dma_start(out=gate_rep_sb[:], in_=gate_rep_ext[:, :])
            eps_sb = singles.tile([128, 1], F32, tag="eps_sb")
            nc.vector.memset(eps_sb[:], EPS)
            ident = singles.tile([128, 128], F32, tag="ident")
            from concourse.masks import make_identity

            make_identity(nc, ident[:])

            # --- weights: f32 -> bf16 -------------------------------------
            wg = {}
            for name, ext in (("v", wv_ext), ("k", wk_ext), ("q", wq_ext)):
                tiles = []
                for kd in range(8):
                    wt = wtmp.tile([128, COLS], F32, tag="wtmp", name=f"wt_{name}{kd}")
                    nc.sync.dma_start(out=wt[:], in_=ext[kd * 128 : (kd + 1) * 128, :])
                    wb = singles.tile(
                        [128, COLS], BF16, tag=f"wg_{name}_{kd}", name=f"wg_{name}{kd}"
                    )
                    nc.vector.tensor_copy(wb[:], wt[:])
                    tiles.append(wb)
                wg[name] = tiles

            # --- LayerNorm (one x tile [128, DIM]) ------------------------
            def ln_tile(src_ext, row0, gate_vec, tag_sfx):
                xt = xin.tile([128, DIM], F32, tag="xin", name=f"x_{tag_sfx}")
                nc.sync.dma_start(out=xt[:], in_=src_ext[row0 : row0 + 128, :])
                st = stats.tile([128, 2, 6], F32, tag="bnst", name=f"st_{tag_sfx}")
                xgr = xt.rearrange("p (s d) -> p s d", s=2)
                nc.vector.bn_stats(out=st[:, 0, :], in_=xgr[:, 0, :])
                nc.vector.bn_stats(out=st[:, 1, :], in_=xgr[:, 1, :])
                mv = stats.tile([128, 2], F32, tag="bnmv", name=f"mv_{tag_sfx}")
                nc.vector.bn_aggr(out=mv[:], in_=st[:])
                stdev = stats.tile([128, 1], F32, tag="stdev", name=f"sd_{tag_sfx}")
                nc.scalar.activation(
                    out=stdev[:],
                    in_=mv[:, 1:2],
                    func=mybir.ActivationFunctionType.Sqrt,
                    bias=eps_sb[:],
                    scale=1.0,
                )
                rstd = stats.tile([128, 1], F32, tag="rstd", name=f"rs_{tag_sfx}")
                nc.vector.reciprocal(out=rstd[:], in_=stdev[:])
                if gate_vec is not None:
                    rstd_g = stats.tile(
                        [128, 1], F32, tag="rstd_g", name=f"rg_{tag_sfx}"
                    )
                    nc.vector.tensor_scalar(
                        out=rstd_g[:], in0=rstd[:], scalar1=gate_vec, scalar2=None,
                        op0=MUL,
                    )
                    rstd = rstd_g
                nmr = stats.tile([128, 1], F32, tag="nmr", name=f"nm_{tag_sfx}")
                nc.vector.tensor_scalar(
                    out=nmr[:], in0=mv[:, 0:1], scalar1=rstd[:], scalar2=-1.0,
                    op0=MUL, op1=MUL,
                )
                xh = xhat_pool.tile([128, DIM], BF16, tag="xhat", name=f"xh_{tag_sfx}")
                nc.vector.tensor_scalar(
                    out=xh[:], in0=xt[:], scalar1=rstd[:], scalar2=nmr[:],
                    op0=MUL, op1=ADD,
                )
                return xh

            # --- kv path: LN -> scratch -> transpose -> affine ------------
            xkvT = [
                singles.tile([128, l_kv], BF16, tag=f"xkvT_{kd}", name=f"xkvT{kd}")
                for kd in range(8)
            ]
            kv_src = xkv_ext if COMPACT else x_ext
            for c, (row0, nrows) in enumerate(kv_chunks):
                for t in range(nrows // 128):
                    tb = (row0 + t * 128) // 128
                    xh = ln_tile(
                        kv_src, row0 + t * 128, gate_sb[:, tb : tb + 1], f"kv{tb}"
                    )
                    nc.sync.dma_start(
                        out=scr_kv[c][t * 128 : (t + 1) * 128, :], in_=xh[:]
                    )
                for kd in range(8):
                    dst = xkvT[kd][:, row0 : row0 + nrows]
                    nc.sync.dma_start_transpose(
                        out=dst, in_=scr_kv[c][:, kd * 128 : (kd + 1) * 128]
                    )
                    nc.vector.tensor_scalar(
                        out=dst, in0=dst,
                        scalar1=g_sb[:, kd : kd + 1], scalar2=b_sb[:, kd : kd + 1],
                        op0=MUL, op1=ADD,
                    )

            # --- q path ----------------------------------------------------
            xqT = [
                singles.tile([128, N_TOK], BF16, tag=f"xqT_{kd}", name=f"xqT{kd}")
                for kd in range(8)
            ]
            for c in range(NQCH):
                for t in range(4):
                    xh = ln_tile(x_ext, c * 512 + t * 128, None, f"q{c}_{t}")
                    nc.sync.dma_start(
                        out=scr_q[c][t * 128 : (t + 1) * 128, :], in_=xh[:]
                    )
                for kd in range(8):
                    dst = xqT[kd][:, c * 512 : (c + 1) * 512]
                    nc.sync.dma_start_transpose(
                        out=dst, in_=scr_q[c][:, kd * 128 : (kd + 1) * 128]
                    )
                    nc.vector.tensor_scalar(
                        out=dst, in0=dst,
                        scalar1=g_sb[:, kd : kd + 1], scalar2=b_sb[:, kd : kd + 1],
                        op0=MUL, op1=ADD,
                    )

            # PSUM tags: s0/s1 = score groups (3 banks each), o0/o1 = AV
            # accumulators / transposes (1 bank each). Projections rotate
            # over all four tags for double buffering. Total 8 banks.
            PROJ_TAGS = ("s0", "s1", "o0", "o1")
            proj_n = [0]

            def proj_psum(n_free, name):
                tag = PROJ_TAGS[proj_n[0] % 4]
                proj_n[0] += 1
                return psum.tile([128, n_free], F32, tag=tag, name=name)

            # --- v projection + vaug --------------------------------------
            vaug = []
            for tb in range(lt):
                va = singles.tile(
                    [128, HEADS_LOCAL * 65], BF16, tag=f"vaug_{tb}", name=f"vaug{tb}"
                )
                ps = proj_psum(COLS, f"psv{tb}")
                for kd in range(8):
                    nc.tensor.matmul(
                        ps[:],
                        xkvT[kd][:, tb * 128 : (tb + 1) * 128],
                        wg["v"][kd][:],
                        start=(kd == 0),
                        stop=(kd == 7),
                    )
                va_r = va.rearrange("p (h c) -> p h c", c=65)
                nc.vector.tensor_copy(
                    va_r[:, :, 0:64], ps.rearrange("p (h c) -> p h c", c=64)
                )
                nc.vector.tensor_copy(
                    va_r[:, :, 64],
                    gate_rep_sb[:, tb * HEADS_LOCAL : (tb + 1) * HEADS_LOCAL],
                )
                vaug.append(va)

            # --- per-head-pair: projections then attention ----------------
            kT = [None] * 4
            qT = [None] * 4
            out_tiles = [
                singles.tile([128, COLS], F32, tag=f"outt_{j}", name=f"outt{j}")
                for j in range(16)
            ]
            ngroups = (lt + KGROUP - 1) // KGROUP

            for hp in range(4):
                cb = hp
                # kT for this column block
                kt = singles.tile([128, l_kv], BF16, tag=f"kT_{cb}", name=f"kT{cb}")
                for row0, nrows in kv_chunks:
                    ps = proj_psum(512, f"psk{cb}_{row0}")
                    for kd in range(8):
                        nc.tensor.matmul(
                            ps[:, :nrows],
                            wg["k"][kd][:, cb * 128 : (cb + 1) * 128],
                            xkvT[kd][:, row0 : row0 + nrows],
                            start=(kd == 0),
                            stop=(kd == 7),
                        )
                    nc.vector.tensor_copy(kt[:, row0 : row0 + nrows], ps[:, :nrows])
                kT[cb] = kt
                qt = singles.tile([128, N_TOK], BF16, tag=f"qT_{cb}", name=f"qT{cb}")
                for tcn in range(4):
                    ps = proj_psum(512, f"psq{cb}_{tcn}")
                    for kd in range(8):
                        nc.tensor.matmul(
                            ps[:],
                            wg["q"][kd][:, cb * 128 : (cb + 1) * 128],
                            xqT[kd][:, tcn * 512 : (tcn + 1) * 512],
                            start=(kd == 0),
                            stop=(kd == 7),
                        )
                    nc.vector.tensor_copy(qt[:, tcn * 512 : (tcn + 1) * 512], ps[:])
                qT[cb] = qt

                h0, h1 = 2 * hp, 2 * hp + 1
                for qc in range(N_TOK // QCHUNK):
                    po = [
                        psum.tile([65, 512], F32, tag=f"o{hh}", name=f"po{hp}_{qc}_{hh}")
                        for hh in range(2)
                    ]
                    for gi in range(ngroups):
                        gsz = min(KGROUP, lt - gi * KGROUP)
                        ps_s = [
                            psum.tile(
                                [128, KGROUP * 512], F32, tag=f"s{hh}",
                                name=f"ps{hp}_{qc}_{gi}_{hh}",
                            )
                            for hh in range(2)
                        ]
                        # Alternate heads so the PE's row groups (0-63 vs
                        # 64-127) overlap weight loads with matmuls.
                        for i in range(gsz):
                            tb = gi * KGROUP + i
                            for hh, p0 in ((0, 0), (1, 64)):
                                nc.tensor.matmul(
                                    ps_s[hh][:, i * 512 : (i + 1) * 512],
                                    kT[cb][p0 : p0 + 64, tb * 128 : (tb + 1) * 128],
                                    qT[cb][p0 : p0 + 64, qc * 512 : (qc + 1) * 512],
                                    start=True,
                                    stop=True,
                                )
                        pp = []
                        for hh in range(2):
                            p_sb = p_pool.tile(
                                [128, KGROUP * 512], BF16, tag=f"p{hh}",
                                name=f"p{hp}_{qc}_{gi}_{hh}",
                            )
                            nc.scalar.activation(
                                out=p_sb[:, : gsz * 512],
                                in_=ps_s[hh][:, : gsz * 512],
                                func=mybir.ActivationFunctionType.Exp,
                                scale=SCALE,
                            )
                            pp.append(p_sb)
                        for i in range(gsz):
                            tb = gi * KGROUP + i
                            for hh, h in ((0, h0), (1, h1)):
                                nc.tensor.matmul(
                                    po[hh][:],
                                    vaug[tb][:, h * 65 : (h + 1) * 65],
                                    pp[hh][:, i * 512 : (i + 1) * 512],
                                    start=(tb == 0),
                                    stop=(tb == lt - 1),
                                )
                    for hh, h in ((0, h0), (1, h1)):
                        o_sb = o_pool.tile(
                            [65, 512], F32, tag="o_sb", name=f"ob{hp}_{qc}_{hh}"
                        )
                        nc.vector.tensor_copy(o_sb[:], po[hh][:])
                        pt = psum.tile(
                            [128, 4 * 65], F32, tag=f"o{hh}", name=f"pt{hp}_{qc}_{hh}"
                        )
                        for j in range(4):
                            nc.tensor.transpose(
                                pt[:, j * 65 : (j + 1) * 65],
                                o_sb[:, j * 128 : (j + 1) * 128],
                                ident[0:65, 0:65],
                            )
                        rc = recip_pool.tile(
                            [128, 4], F32, tag="recip", name=f"rc{hp}_{qc}_{hh}"
                        )
                        nc.vector.reciprocal(
                            out=rc[:],
                            in_=pt.rearrange("p (j c) -> p j c", c=65)[:, :, 64:65],
                        )
                        for j in range(4):
                            nc.vector.tensor_scalar(
                                out=out_tiles[qc * 4 + j][:, h * 64 : (h + 1) * 64],
                                in0=pt[:, j * 65 : j * 65 + 64],
                                scalar1=rc[:, j : j + 1],
                                scalar2=None,
                                op0=MUL,
                            )

            for jt in range(16):
                nc.sync.dma_start(
                    out=out_ext[jt * 128 : (jt + 1) * 128, :], in_=out_tiles[jt][:]
                )

    _split_excess_waits(nc)
    return nc


_GRAPH_CACHE = {}


def kernel(x, mask, w_qkv, ln_g, ln_b):
    x = np.asarray(x, dtype=np.float32)
    mask = np.asarray(mask)
    w_qkv = np.asarray(w_qkv, dtype=np.float32)
    ln_g = np.asarray(ln_g, dtype=np.float32)
    ln_b = np.asarray(ln_b, dtype=np.float32)
    b, n, d = x.shape

    if COMPACT:
        keeps = [np.where(mask[bi] == 0)[0] for bi in range(b)]
        l_kv = max(128, -(-max(len(k) for k in keeps) // 128) * 128)
    else:
        keeps = None
        l_kv = n
    lt = l_kv // 128

    global LAST_EXEC_TIME_NS
    key = (l_kv, COMPACT)
    if key not in _GRAPH_CACHE:
        _GRAPH_CACHE[key] = build_graph(l_kv)
    nc = _GRAPH_CACHE[key]

    in_maps = []
    for core in range(N_CORES):
        bi, hg = core // 2, core % 2
        if COMPACT:
            keep = keeps[bi]
            xkv = np.zeros((l_kv, d), dtype=np.float32)
            xkv[: len(keep)] = x[bi][keep]
            gate = np.zeros((l_kv,), dtype=np.float32)
            gate[: len(keep)] = 1.0
        else:
            gate = 1.0 - mask[bi].astype(np.float32)
        gate_rep = np.repeat(
            gate.reshape(lt, 128).T[:, :, None], HEADS_LOCAL, axis=2
        ).reshape(128, lt * HEADS_LOCAL)
        m = {
            "x": x[bi],
            "gate": gate,
            "gate_rep": np.ascontiguousarray(gate_rep),
            "wq": np.ascontiguousarray(w_qkv[:, hg * COLS : (hg + 1) * COLS]),
            "wk": np.ascontiguousarray(w_qkv[:, d + hg * COLS : d + (hg + 1) * COLS]),
            "wv": np.ascontiguousarray(
                w_qkv[:, 2 * d + hg * COLS : 2 * d + (hg + 1) * COLS]
            ),
            "ln_g": ln_g,
            "ln_b": ln_b,
        }
        if COMPACT:
            m["xkv"] = xkv
        in_maps.append(m)

    from concourse.bass_utils import run_bass_kernel_spmd

    trace = os.environ.get("KERNEL_TRACE", "") == "1"
    kwargs = {}
    if trace:
        import antenv

        if "antenv.axon_hooks" not in sys.modules:
            hooks = types.ModuleType("antenv.axon_hooks")
            hooks._hook = None
            hooks.set_axon_ntff_profile_hook = lambda h: setattr(hooks, "_hook", h)
            hooks.get_axon_ntff_profile_hook = lambda: hooks._hook
            sys.modules["antenv.axon_hooks"] = hooks
            antenv.axon_hooks = hooks
        from trn_agent_boot.trn_boot import _ntff_profile_via_ctypes

        sys.modules["antenv.axon_hooks"].set_axon_ntff_profile_hook(
            _ntff_profile_via_ctypes("/opt/axon/libaxon_pjrt.so")
        )
        from concourse import bass_utils

        bass_utils.upload_artifacts = lambda tmpdir: tmpdir
        import uuid

        tdir = os.path.join(
            os.environ.get("KERNEL_TRACE_DIR", "/tmp/kernel_trace"),
            uuid.uuid4().hex[:8],
        )
        os.makedirs(tdir, exist_ok=True)
        kwargs = {"trace": True, "tmpdir": tdir}

    res = run_bass_kernel_spmd(nc, in_maps, core_ids=list(range(N_CORES)), **kwargs)
    LAST_EXEC_TIME_NS = res.exec_time_ns

    out = np.empty((b, n, d), dtype=np.float32)
    for core in range(N_CORES):
        bi, hg = core // 2, core % 2
        out[bi][:, hg * COLS : (hg + 1) * COLS] = res.results[core]["out"]
    return out


# revision 12
# speedup vs baseline: 1.0441x; 1.0441x over previous
"""Distributed Trainium2 kernel for nn_Attention_49529562858354.

Reference computation (per batch): LayerNorm(x) @ w_qkv -> 16-head
self-attention with key-side masking (mask==1 -> key excluded).

Sharding (8 cores): core = batch * 2 + head_group. Data parallel over
the 4 batches, tensor parallel over 2 groups of 8 heads. Each core gets
its batch's x, the w_qkv column slice for its heads, and produces
out[:, hg*512:(hg+1)*512] for its batch. No collectives needed.

Per-core pipeline (bf16 compute, f32 LN stats):
  1. LayerNorm in natural layout (bn_stats; rstd = exp(-0.5*ln(var+eps))
     so the ScalarE stays on one activation-table set with attention's
     exp). kv rows also fold in the key gate (masked/pad keys -> zero
     rows, which zeroes their V rows and denominator entries
     downstream). xhat -> bf16 -> DRAM scratch -> DMA-transpose ->
     xT [d, tokens] tiles. ln_g is folded into the weight cast; ln_b
     (zero in this problem) gets a fallback affine pass on xT.
  2. QKV projections: qT/kT as [cols, tokens], v natural with a gate
     column per head (softmax denominator comes out of the AV matmul).
  3. Attention, q-chunk outer / head-pair inner so the first chunk's
     scores start while later chunks are still in LayerNorm: scores
     transposed [k, q] in PSUM, exp on ScalarE with fused scale (no max
     subtraction: post-LN logits are O(1)), AV accumulates [65, 512]
     with row 64 = denominator. Epilogue: PE transpose, reciprocal,
     scale, one [128, 512] out tile per 128 queries.

Masked keys are removed on the host (gather) and padded to a multiple
of 128 with gate=0 rows, roughly halving attention work. Set
KERNEL_DENSE=1 to run dense (all 2048 keys, gate = 1-mask).
"""

import os
import sys
import types

for _p in ("/opt/trn_rl_repo", "/root/.axon_site"):
    if _p not in sys.path:
        sys.path.insert(0, _p)

import numpy as np
import ml_dtypes

import concourse.bass as bass
import concourse.tile as tile
from concourse import mybir

N_CORES = 8
N_TOK = 2048
DIM = 1024
HEADS_LOCAL = 8
DH = 64
COLS = HEADS_LOCAL * DH  # 512 columns per core per q/k/v
SCALE = DH ** -0.5
EPS = 1e-5
QCHUNK = 512
KGROUP = 3  # score k-tiles per PSUM group / exp call
COMPACT = os.environ.get("KERNEL_DENSE", "") != "1"

F32 = mybir.dt.float32
BF16 = mybir.dt.bfloat16
MUL = mybir.AluOpType.mult
ADD = mybir.AluOpType.add

LAST_EXEC_TIME_NS = None


def _split_excess_waits(nc, max_waits=1, max_updates=1):
    """This container's walrus rejects >1 sync wait/update per
    instruction; move overflow onto adjacent same-engine NoOps."""
    counter = [0]

    def fresh():
        counter[0] += 1
        return f"I-WFIX-{counter[0]}"

    for f in nc.m.functions:
        for blk in f.blocks:
            il = blk.instructions
            out = []
            changed = False
            for inst in il:
                si = inst.sync_info
                if si is None:
                    out.append(inst)
                    continue
                waits = list(si.on_wait or [])
                updates = list(si.on_update or [])
                pre, post = [], []
                if len(waits) > max_waits:
                    for w in waits[max_waits:]:
                        nop = mybir.InstNoOp(name=fresh(), ins=[], outs=[])
                        nop.engine = inst.engine
                        nop.sync_info = mybir.SyncInfo(on_wait=[w], on_update=[])
                        pre.append(nop)
                    waits = waits[:max_waits]
                if len(updates) > max_updates:
                    for u in updates[max_updates:]:
                        nop = mybir.InstNoOp(name=fresh(), ins=[], outs=[])
                        nop.engine = inst.engine
                        nop.sync_info = mybir.SyncInfo(on_wait=[], on_update=[u])
                        post.append(nop)
                    updates = updates[:max_updates]
                if pre or post:
                    inst.sync_info = mybir.SyncInfo(on_wait=waits, on_update=updates)
                    changed = True
                out.extend(pre)
                out.append(inst)
                out.extend(post)
            if changed:
                blk.instructions = out


def build_graph(l_kv, has_bias):
    lt = l_kv // 128  # kv token tiles
    nc = bass.Bass()

    x_ext = nc.declare_dram_parameter("x", [N_TOK, DIM], F32, isOutput=False)
    xkv_ext = (
        nc.declare_dram_parameter("xkv", [l_kv, DIM], F32, isOutput=False)
        if COMPACT
        else None
    )
    gate_ext = nc.declare_dram_parameter("gate", [l_kv], F32, isOutput=False)
    gate_rep_ext = nc.declare_dram_parameter(
        "gate_rep", [128, lt * HEADS_LOCAL], F32, isOutput=False
    )
    wq_ext = nc.declare_dram_parameter("wq", [DIM, COLS], F32, isOutput=False)
    wk_ext = nc.declare_dram_parameter("wk", [DIM, COLS], F32, isOutput=False)
    wv_ext = nc.declare_dram_parameter("wv", [DIM, COLS], F32, isOutput=False)
    g_ext = nc.declare_dram_parameter("ln_g", [DIM], F32, isOutput=False)
    b_ext = nc.declare_dram_parameter("ln_b", [DIM], F32, isOutput=False)
    out_ext = nc.declare_dram_parameter("out", [N_TOK, COLS], F32, isOutput=True)

    # Row-chunked scratch so DMA-transposes pipeline with the LN.
    NQCH = N_TOK // QCHUNK  # 4 q chunks x 4 tiles
    kv_chunks = [(0, l_kv)]
    scr_q = [nc.dram_tensor(f"scr_q{c}", [QCHUNK, DIM], BF16) for c in range(NQCH)]
    scr_kv = [
        nc.dram_tensor(f"scr_kv{c}", [nr, DIM], BF16)
        for c, (_, nr) in enumerate(kv_chunks)
    ]

    with tile.TileContext(nc) as tc:
        import contextlib

        with contextlib.ExitStack() as ctx:
            singles = ctx.enter_context(tc.tile_pool(name="singles", bufs=1))
            xin = ctx.enter_context(tc.tile_pool(name="xin", bufs=3))
            stats = ctx.enter_context(tc.tile_pool(name="stats", bufs=3))
            xhat_pool = ctx.enter_context(tc.tile_pool(name="xhat", bufs=3))
            wtmp = ctx.enter_context(tc.tile_pool(name="wtmp", bufs=2))
            psum = ctx.enter_context(tc.tile_pool(name="psum", bufs=1, space="PSUM"))
            p_pool = ctx.enter_context(tc.tile_pool(name="p_sb", bufs=2))
            o_pool = ctx.enter_context(tc.tile_pool(name="o_sb", bufs=2))
            out_pool = ctx.enter_context(tc.tile_pool(name="outt", bufs=2))
            recip_pool = ctx.enter_context(tc.tile_pool(name="recip", bufs=2))

            # --- constants -------------------------------------------------
            g_sb = singles.tile([128, 8], F32, tag="g_sb")
            nc.sync.dma_start(out=g_sb[:], in_=g_ext.rearrange("(kd p) -> p kd", p=128))
            b_sb = singles.tile([128, 8], F32, tag="b_sb")
            nc.sync.dma_start(out=b_sb[:], in_=b_ext.rearrange("(kd p) -> p kd", p=128))
            gate_sb = singles.tile([128, lt], F32, tag="gate_sb")
            nc.sync.dma_start(
                out=gate_sb[:], in_=gate_ext.rearrange("(t p) -> p t", p=128)
            )
            gate_rep_sb = singles.tile([128, lt * HEADS_LOCAL], F32, tag="gate_rep_sb")
            nc.sync.dma_start(out=gate_rep_sb[:], in_=gate_rep_ext[:, :])
            eps_sb = singles.tile([128, 1], F32, tag="eps_sb")
            nc.vector.memset(eps_sb[:], EPS)
            ident = singles.tile([128, 128], F32, tag="ident")
            from concourse.masks import make_identity

            make_identity(nc, ident[:])

            # --- weights: f32 -> bf16, ln_g folded in (per-partition d) ---
            wg = {}
            for name, ext in (("v", wv_ext), ("k", wk_ext), ("q", wq_ext)):
                tiles = []
                for kd in range(8):
                    wt = wtmp.tile([128, COLS], F32, tag="wtmp", name=f"wt_{name}{kd}")
                    nc.sync.dma_start(out=wt[:], in_=ext[kd * 128 : (kd + 1) * 128, :])
                    wb = singles.tile(
                        [128, COLS], BF16, tag=f"wg_{name}_{kd}", name=f"wg_{name}{kd}"
                    )
                    nc.vector.tensor_scalar(
                        out=wb[:], in0=wt[:], scalar1=g_sb[:, kd : kd + 1],
                        scalar2=None, op0=MUL,
                    )
                    tiles.append(wb)
                wg[name] = tiles

            # --- LayerNorm for one x tile [128, DIM] ----------------------
            def ln_tile(src_ext, row0, gate_vec, tag_sfx):
                xt = xin.tile([128, DIM], F32, tag="xin", name=f"x_{tag_sfx}")
                nc.gpsimd.dma_start(out=xt[:], in_=src_ext[row0 : row0 + 128, :])
                st = stats.tile([128, 2, 6], F32, tag="bnst", name=f"st_{tag_sfx}")
                xgr = xt.rearrange("p (s d) -> p s d", s=2)
                nc.vector.bn_stats(out=st[:, 0, :], in_=xgr[:, 0, :])
                nc.vector.bn_stats(out=st[:, 1, :], in_=xgr[:, 1, :])
                mv = stats.tile([128, 2], F32, tag="bnmv", name=f"mv_{tag_sfx}")
                nc.vector.bn_aggr(out=mv[:], in_=st[:])
                # rstd = exp(-0.5*ln(var+eps)) — same ACT table set as the
                # attention exp, so no table reloads.
                lv = stats.tile([128, 1], F32, tag="lv", name=f"lv_{tag_sfx}")
                nc.scalar.activation(
                    out=lv[:],
                    in_=mv[:, 1:2],
                    func=mybir.ActivationFunctionType.Ln,
                    bias=eps_sb[:],
                    scale=1.0,
                )
                rstd = stats.tile([128, 1], F32, tag="rstd", name=f"rs_{tag_sfx}")
                nc.scalar.activation(
                    out=rstd[:],
                    in_=lv[:],
                    func=mybir.ActivationFunctionType.Exp,
                    scale=-0.5,
                )
                if gate_vec is not None:
                    rstd_g = stats.tile(
                        [128, 1], F32, tag="rstd_g", name=f"rg_{tag_sfx}"
                    )
                    nc.vector.tensor_scalar(
                        out=rstd_g[:], in0=rstd[:], scalar1=gate_vec, scalar2=None,
                        op0=MUL,
                    )
                    rstd = rstd_g
                nmr = stats.tile([128, 1], F32, tag="nmr", name=f"nm_{tag_sfx}")
                nc.vector.tensor_scalar(
                    out=nmr[:], in0=mv[:, 0:1], scalar1=rstd[:], scalar2=-1.0,
                    op0=MUL, op1=MUL,
                )
                xh = xhat_pool.tile([128, DIM], BF16, tag="xhat", name=f"xh_{tag_sfx}")
                nc.vector.tensor_scalar(
                    out=xh[:], in0=xt[:], scalar1=rstd[:], scalar2=nmr[:],
                    op0=MUL, op1=ADD,
                )
                return xh

            def prep_chunk(src_ext, row0, nrows, scratch, xT_tiles, gated, pfx):
                for t in range(nrows // 128):
                    tb = (row0 + t * 128) // 128
                    gv = gate_sb[:, tb : tb + 1] if gated else None
                    xh = ln_tile(src_ext, row0 + t * 128, gv, f"{pfx}{tb}")
                    nc.scalar.dma_start(
                        out=scratch[t * 128 : (t + 1) * 128, :], in_=xh[:]
                    )
                for kd in range(8):
                    dst = xT_tiles[kd][:, row0 : row0 + nrows]
                    nc.sync.dma_start_transpose(
                        out=dst, in_=scratch[:, kd * 128 : (kd + 1) * 128]
                    )
                    if has_bias:
                        nc.vector.tensor_scalar(
                            out=dst, in0=dst, scalar1=b_sb[:, kd : kd + 1],
                            scalar2=None, op0=ADD,
                        )

            # --- kv path, then the first q chunk ---------------------------
            xkvT = [
                singles.tile([128, l_kv], BF16, tag=f"xkvT_{kd}", name=f"xkvT{kd}")
                for kd in range(8)
            ]
            kv_src = xkv_ext if COMPACT else x_ext
            for c, (row0, nrows) in enumerate(kv_chunks):
                prep_chunk(kv_src, row0, nrows, scr_kv[c], xkvT, True, "kv")

            xqT = [
                singles.tile([128, N_TOK], BF16, tag=f"xqT_{kd}", name=f"xqT{kd}")
                for kd in range(8)
            ]
            prep_chunk(x_ext, 0, QCHUNK, scr_q[0], xqT, False, "q")

            # PSUM tags: s0/s1 = score groups (3 banks each), o0/o1 = AV
            # accumulators / epilogue transposes (1 bank each). Projections
            # rotate over all four tags. Total 8 banks.
            PROJ_TAGS = ("s0", "s1", "o0", "o1")
            proj_n = [0]

            def proj_psum(n_free, name):
                tag = PROJ_TAGS[proj_n[0] % 4]
                proj_n[0] += 1
                return psum.tile([128, n_free], F32, tag=tag, name=name)

            # --- v projection + vaug (gate already folded into xhat_kv) ---
            vaug = []
            for tb in range(lt):
                va = singles.tile(
                    [128, HEADS_LOCAL * 65], BF16, tag=f"vaug_{tb}", name=f"vaug{tb}"
                )
                ps = proj_psum(COLS, f"psv{tb}")
                for kd in range(8):
                    nc.tensor.matmul(
                        ps[:],
                        xkvT[kd][:, tb * 128 : (tb + 1) * 128],
                        wg["v"][kd][:],
                        start=(kd == 0),
                        stop=(kd == 7),
                    )
                va_r = va.rearrange("p (h c) -> p h c", c=65)
                nc.vector.tensor_copy(
                    va_r[:, :, 0:64], ps.rearrange("p (h c) -> p h c", c=64)
                )
                nc.vector.tensor_copy(
                    va_r[:, :, 64],
                    gate_rep_sb[:, tb * HEADS_LOCAL : (tb + 1) * HEADS_LOCAL],
                )
                vaug.append(va)

            # --- kT projections (all column blocks) ------------------------
            kT = []
            kproj_chunks = []
            off = 0
            while off < l_kv:
                sz = min(512, l_kv - off)
                kproj_chunks.append((off, sz))
                off += sz
            for cb in range(4):
                kt = singles.tile([128, l_kv], BF16, tag=f"kT_{cb}", name=f"kT{cb}")
                for row0, nrows in kproj_chunks:
                    ps = proj_psum(512, f"psk{cb}_{row0}")
                    for kd in range(8):
                        nc.tensor.matmul(
                            ps[:, :nrows],
                            wg["k"][kd][:, cb * 128 : (cb + 1) * 128],
                            xkvT[kd][:, row0 : row0 + nrows],
                            start=(kd == 0),
                            stop=(kd == 7),
                        )
                    nc.vector.tensor_copy(kt[:, row0 : row0 + nrows], ps[:, :nrows])
                kT.append(kt)

            # --- attention: q-chunk outer, head-pair inner ------------------
            qT = [
                singles.tile([128, N_TOK], BF16, tag=f"qT_{cb}", name=f"qT{cb}")
                for cb in range(4)
            ]
            ngroups = (lt + KGROUP - 1) // KGROUP

            for qc in range(NQCH):
                # qT projection for this token chunk, all column blocks.
                for cb in range(4):
                    ps = proj_psum(512, f"psq{cb}_{qc}")
                    for kd in range(8):
                        nc.tensor.matmul(
                            ps[:],
                            wg["q"][kd][:, cb * 128 : (cb + 1) * 128],
                            xqT[kd][:, qc * 512 : (qc + 1) * 512],
                            start=(kd == 0),
                            stop=(kd == 7),
                        )
                    nc.vector.tensor_copy(qT[cb][:, qc * 512 : (qc + 1) * 512], ps[:])

                # prefetch next q chunk's LayerNorm while attention runs
                if qc + 1 < NQCH:
                    prep_chunk(
                        x_ext, (qc + 1) * QCHUNK, QCHUNK, scr_q[qc + 1], xqT, False, "q"
                    )

                out_tiles = [
                    out_pool.tile([128, COLS], F32, tag=f"out_{j}", name=f"o{qc}_{j}")
                    for j in range(4)
                ]
                for hp in range(4):
                    cb = hp
                    h0, h1 = 2 * hp, 2 * hp + 1
                    po = [
                        psum.tile(
                            [65, 512], F32, tag=f"o{hh}", name=f"po{qc}_{hp}_{hh}"
                        )
                        for hh in range(2)
                    ]
                    for gi in range(ngroups):
                        gsz = min(KGROUP, lt - gi * KGROUP)
                        ps_s = [
                            psum.tile(
                                [128, KGROUP * 512], F32, tag=f"s{hh}",
                                name=f"ps{qc}_{hp}_{gi}_{hh}",
                            )
                            for hh in range(2)
                        ]
                        for i in range(gsz):
                            tb = gi * KGROUP + i
                            for hh, p0 in ((0, 0), (1, 64)):
                                nc.tensor.matmul(
                                    ps_s[hh][:, i * 512 : (i + 1) * 512],
                                    kT[cb][p0 : p0 + 64, tb * 128 : (tb + 1) * 128],
                                    qT[cb][p0 : p0 + 64, qc * 512 : (qc + 1) * 512],
                                    start=True,
                                    stop=True,
                                )
                        pp = []
                        for hh in range(2):
                            p_sb = p_pool.tile(
                                [128, KGROUP * 512], BF16, tag=f"p{hh}",
                                name=f"p{qc}_{hp}_{gi}_{hh}",
                            )
                            nc.scalar.activation(
                                out=p_sb[:, : gsz * 512],
                                in_=ps_s[hh][:, : gsz * 512],
                                func=mybir.ActivationFunctionType.Exp,
                                scale=SCALE,
                            )
                            pp.append(p_sb)
                        for i in range(gsz):
                            tb = gi * KGROUP + i
                            for hh, h in ((0, h0), (1, h1)):
                                nc.tensor.matmul(
                                    po[hh][:],
                                    vaug[tb][:, h * 65 : (h + 1) * 65],
                                    pp[hh][:, i * 512 : (i + 1) * 512],
                                    start=(tb == 0),
                                    stop=(tb == lt - 1),
                                )
                    for hh, h in ((0, h0), (1, h1)):
                        o_sb = o_pool.tile(
                            [65, 512], F32, tag="o_sb", name=f"ob{qc}_{hp}_{hh}"
                        )
                        nc.vector.tensor_copy(o_sb[:], po[hh][:])
                        pt = psum.tile(
                            [128, 4 * 65], F32, tag=f"o{hh}", name=f"pt{qc}_{hp}_{hh}"
                        )
                        for j in range(4):
                            nc.tensor.transpose(
                                pt[:, j * 65 : (j + 1) * 65],
                                o_sb[:, j * 128 : (j + 1) * 128],
                                ident[0:65, 0:65],
                            )
                        rc = recip_pool.tile(
                            [128, 4], F32, tag="recip", name=f"rc{qc}_{hp}_{hh}"
                        )
                        nc.vector.reciprocal(
                            out=rc[:],
                            in_=pt.rearrange("p (j c) -> p j c", c=65)[:, :, 64:65],
                        )
                        for j in range(4):
                            nc.vector.tensor_scalar(
                                out=out_tiles[j][:, h * 64 : (h + 1) * 64],
                                in0=pt[:, j * 65 : j * 65 + 64],
                                scalar1=rc[:, j : j + 1],
                                scalar2=None,
                                op0=MUL,
                            )
                for j in range(4):
                    row0 = qc * QCHUNK + j * 128
                    nc.sync.dma_start(
                        out=out_ext[row0 : row0 + 128, :], in_=out_tiles[j][:]
                    )

    _split_excess_waits(nc)
    return nc


_GRAPH_CACHE = {}


def kernel(x, mask, w_qkv, ln_g, ln_b):
    x = np.asarray(x, dtype=np.float32)
    mask = np.asarray(mask)
    w_qkv = np.asarray(w_qkv, dtype=np.float32)
    ln_g = np.asarray(ln_g, dtype=np.float32)
    ln_b = np.asarray(ln_b, dtype=np.float32)
    b, n, d = x.shape

    if COMPACT:
        keeps = [np.where(mask[bi] == 0)[0] for bi in range(b)]
        l_kv = max(128, -(-max(len(k) for k in keeps) // 128) * 128)
    else:
        keeps = None
        l_kv = n
    lt = l_kv // 128
    has_bias = bool(np.any(ln_b != 0.0))

    global LAST_EXEC_TIME_NS
    key = (l_kv, COMPACT, has_bias)
    if key not in _GRAPH_CACHE:
        _GRAPH_CACHE[key] = build_graph(l_kv, has_bias)
    nc = _GRAPH_CACHE[key]

    in_maps = []
    for core in range(N_CORES):
        bi, hg = core // 2, core % 2
        if COMPACT:
            keep = keeps[bi]
            xkv = np.zeros((l_kv, d), dtype=np.float32)
            xkv[: len(keep)] = x[bi][keep]
            gate = np.zeros((l_kv,), dtype=np.float32)
            gate[: len(keep)] = 1.0
        else:
            gate = 1.0 - mask[bi].astype(np.float32)
        gate_rep = np.repeat(
            gate.reshape(lt, 128).T[:, :, None], HEADS_LOCAL, axis=2
        ).reshape(128, lt * HEADS_LOCAL)
        m = {
            "x": x[bi],
            "gate": gate,
            "gate_rep": np.ascontiguousarray(gate_rep),
            "wq": np.ascontiguousarray(w_qkv[:, hg * COLS : (hg + 1) * COLS]),
            "wk": np.ascontiguousarray(w_qkv[:, d + hg * COLS : d + (hg + 1) * COLS]),
            "wv": np.ascontiguousarray(
                w_qkv[:, 2 * d + hg * COLS : 2 * d + (hg + 1) * COLS]
            ),
            "ln_g": ln_g,
            "ln_b": ln_b,
        }
        if COMPACT:
            m["xkv"] = xkv
        in_maps.append(m)

    from concourse.bass_utils import run_bass_kernel_spmd

    trace = os.environ.get("KERNEL_TRACE", "") == "1"
    kwargs = {}
    if trace:
        import antenv

        if "antenv.axon_hooks" not in sys.modules:
            hooks = types.ModuleType("antenv.axon_hooks")
            hooks._hook = None
            hooks.set_axon_ntff_profile_hook = lambda h: setattr(hooks, "_hook", h)
            hooks.get_axon_ntff_profile_hook = lambda: hooks._hook
            sys.modules["antenv.axon_hooks"] = hooks
            antenv.axon_hooks = hooks
        from trn_agent_boot.trn_boot import _ntff_profile_via_ctypes

        sys.modules["antenv.axon_hooks"].set_axon_ntff_profile_hook(
            _ntff_profile_via_ctypes("/opt/axon/libaxon_pjrt.so")
        )
        from concourse import bass_utils

        bass_utils.upload_artifacts = lambda tmpdir: tmpdir
        import uuid

        tdir = os.path.join(
            os.environ.get("KERNEL_TRACE_DIR", "/tmp/kernel_trace"),
            uuid.uuid4().hex[:8],
        )
        os.makedirs(tdir, exist_ok=True)
        kwargs = {"trace": True, "tmpdir": tdir}

    res = run_bass_kernel_spmd(nc, in_maps, core_ids=list(range(N_CORES)), **kwargs)
    LAST_EXEC_TIME_NS = res.exec_time_ns

    out = np.empty((b, n, d), dtype=np.float32)
    for core in range(N_CORES):
        bi, hg = core // 2, core % 2
        out[bi][:, hg * COLS : (hg + 1) * COLS] = res.results[core]["out"]
    return out
